# revision 3
# baseline (speedup 1.0000x reference)
"""MixerAttention (GQA + QK-RMSNorm + RoPE + causal) Trainium2 kernel.

Sharding: 8 cores = batch(2) x kv-head(4). Fully local per core — no collectives.
Each core computes, for its (batch b, kv head h):
  - q/k/v projections for its 4 q heads + 1 kv head (contraction over D on
    the PE, inputs pre-transposed on the host so D lands on partitions)
  - QK RMSNorm (ln/exp rsqrt path) + RoPE (DVE), with the 1/sqrt(DH)
    attention scale folded into the q normalization
  - causal attention in S^T layout: scores^T tiles (t_k partitions x t_q
    free) feed exp (ACT) and then P@V directly as the matmul moving operand;
    softmax denominators ride a ones-matmul; normalization on-chip via
    reciprocal_approx_fast
Matmuls run in float32r (rounded fp32, full PE rate at N>=256).
Output per core is y^T (4*128, T); the host reassembles (B, T, H*DH).
"""
import sys

sys.path.insert(0, "/opt/trn_rl_repo")
import numpy as np
import concourse.bacc as bacc
import concourse.mybir as mybir
import concourse.tile as tile
from concourse.bass_utils import run_bass_kernel_spmd
from concourse.masks import make_identity

F32 = mybir.dt.float32
F32R = mybir.dt.float32r
AF = mybir.ActivationFunctionType

B, T, D = 2, 2048, 2048
H, HKV, DH = 16, 4, 128
G = H // HKV                    # q heads per kv head (per core)
EPS = 1.1920928955078125e-07
ROPE_BASE = 10000.0
NCORES = 8

P = 128                         # partitions
DCH = D // P                    # 16 d-chunks (contraction)
NT = 4                          # t-chunks of 512 for projections
TC = T // NT                    # 512
EQ, EK, EV = G * DH, DH, DH     # 512, 128, 128
ETOT = EQ + EK + EV             # 768
MT = ETOT // P                  # 6 output row tiles
QC = 512                        # attention q-chunk
NQC = T // QC                   # 4
KC = 128                        # attention k-chunk
NKC = T // KC                   # 16
NEG = -1.0e30


def build_nc():
    nc = bacc.Bacc(trn_type="TRN2")
    XT = nc.dram_tensor("xT", [DCH, P, T], F32, kind="ExternalInput")
    WT = nc.dram_tensor("wT", [DCH, P, ETOT], F32, kind="ExternalInput")
    RC = nc.dram_tensor("ropeC", [P, T], F32, kind="ExternalInput")
    RS_ = nc.dram_tensor("ropeS", [P, T], F32, kind="ExternalInput")
    TRI = nc.dram_tensor("trineg", [P, KC], F32, kind="ExternalInput")
    YT = nc.dram_tensor("yT", [EQ, T], F32, kind="ExternalOutput")

    with tile.TileContext(nc) as tc:
        from contextlib import ExitStack

        with ExitStack() as ctx:
            constp = ctx.enter_context(tc.tile_pool(name="const", bufs=1))
            finp = ctx.enter_context(tc.tile_pool(name="final", bufs=1))

            trineg = constp.tile([P, KC], F32, tag="trineg")
            nc.sync.dma_start(out=trineg, in_=TRI[:, :])
            ident = constp.tile([P, P], F32, tag="ident")
            make_identity(nc, ident)
            ones_f = constp.tile([P, P], F32, tag="ones_f")
            nc.vector.memset(ones_f, 1.0)
            ones_r = constp.tile([P, P], F32R, tag="ones_r")
            nc.vector.tensor_copy(ones_r, ones_f)
            bq = constp.tile([P, 1], F32, tag="bq")
            nc.vector.memset(bq, 128.0 * EPS)
            bk = constp.tile([P, 1], F32, tag="bk")
            nc.vector.memset(bk, EPS)
            ropeC = constp.tile([P, T], F32, tag="ropeC")
            nc.sync.dma_start(out=ropeC, in_=RC[:, :])
            ropeS = constp.tile([P, T], F32, tag="ropeS")
            nc.sync.dma_start(out=ropeS, in_=RS_[:, :])

            # final operands of the attention matmuls
            QTr = finp.tile([P, G * T], F32R, tag="QTr")     # 4 heads side by side
            KTr = finp.tile([P, T], F32R, tag="KTr")
            Vnat = finp.tile([P, NKC, KC], F32R, tag="Vnat")  # V in (t_k, d) layout

            with ExitStack() as mid:
                midp = mid.enter_context(tc.tile_pool(name="mid", bufs=1))
                QT = midp.tile([P, G * T], F32, tag="QT")
                KT = midp.tile([P, T], F32, tag="KT")
                VTt = midp.tile([P, T], F32, tag="VTt")

                # ---- Phase A: projections  q^T/k^T/v^T = W^T.T @ x^T ----
                with ExitStack() as prj:
                    wp = prj.enter_context(tc.tile_pool(name="wp", bufs=1))
                    xp = prj.enter_context(tc.tile_pool(name="xp", bufs=3))
                    pp = prj.enter_context(
                        tc.tile_pool(name="pp", bufs=7, space="PSUM")
                    )
                    wt = wp.tile([P, DCH, ETOT], F32R, tag="wt")
                    # one SWDGE cast DMA, (d,p,e) -> (p,d,e)
                    nc.gpsimd.dma_start(
                        out=wt,
                        in_=WT.rearrange("d p e -> p d e"),
                    )
                    for n in range(NT):
                        psl = [
                            pp.tile([P, TC], F32, tag="pj", name=f"pj_{n}_{m}")
                            for m in range(MT)
                        ]
                        for d in range(DCH):
                            xn = xp.tile([P, TC], F32R, tag="xn")
                            nc.gpsimd.dma_start(
                                out=xn, in_=XT[d, :, n * TC : (n + 1) * TC]
                            )
                            for m in range(MT):
                                nc.tensor.matmul(
                                    psl[m],
                                    wt[:, d, m * P : (m + 1) * P],
                                    xn,
                                    start=(d == 0),
                                    stop=(d == DCH - 1),
                                )
                        for m in range(G):
                            nc.scalar.copy(
                                QT[:, m * T + n * TC : m * T + (n + 1) * TC], psl[m]
                            )
                        nc.scalar.copy(KT[:, n * TC : (n + 1) * TC], psl[G])
                        nc.scalar.copy(VTt[:, n * TC : (n + 1) * TC], psl[G + 1])

                # ---- Phase B: RMSNorm + RoPE (q heads and k), V transpose ----
                with ExitStack() as rp:
                    sp = rp.enter_context(tc.tile_pool(name="sp", bufs=2))
                    sp1 = rp.enter_context(tc.tile_pool(name="sp1", bufs=1))
                    rpp = rp.enter_context(
                        tc.tile_pool(name="rpp", bufs=2, space="PSUM")
                    )

                    groups = [(QT, g * T, QTr, g * T, 1.0, bq) for g in range(G)]
                    groups.append((KT, 0, KTr, 0, 1.0 / P, bk))
                    for src, c0, dst, o0, ln_scale, ln_bias in groups:
                        sq = sp.tile([P, T], F32R, tag="sq")
                        nc.scalar.activation(
                            sq, src[:, c0 : c0 + T], AF.Square
                        )
                        ssb = rpp.tile([P, T], F32, tag="ssb", bufs=1)
                        for c in range(NT):
                            nc.tensor.matmul(
                                ssb[:, c * TC : (c + 1) * TC],
                                ones_r,
                                sq[:, c * TC : (c + 1) * TC],
                                start=True,
                                stop=True,
                            )
                        lnt = sp1.tile([P, T], F32, tag="lnt")
                        nc.scalar.activation(
                            lnt, ssb, AF.Ln, scale=ln_scale, bias=ln_bias[:, :]
                        )
                        rs = sp.tile([P, T], F32, tag="rs")
                        nc.scalar.activation(rs, lnt, AF.Exp, scale=-0.5)
                        # normalize in place
                        nc.vector.tensor_mul(
                            src[:, c0 : c0 + T], src[:, c0 : c0 + T], rs
                        )
                        # rope: dst = x*C + rot(x)*S
                        tmp = sp1.tile([P, T], F32, tag="rtmp")
                        nc.vector.tensor_copy(
                            tmp[0 : P // 2, :], src[P // 2 : P, c0 : c0 + T]
                        )
                        nc.vector.tensor_copy(
                            tmp[P // 2 : P, :], src[0 : P // 2, c0 : c0 + T]
                        )
                        t1 = sp1.tile([P, T], F32, tag="rt1")
                        nc.vector.tensor_mul(t1, src[:, c0 : c0 + T], ropeC)
                        nc.vector.tensor_mul(tmp, tmp, ropeS)
                        nc.vector.tensor_add(dst[:, o0 : o0 + T], t1, tmp)

                    # V -> natural layout via PE transposes
                    for j in range(NKC):
                        vps = rpp.tile([P, KC], F32, tag="vps")
                        nc.tensor.transpose(
                            vps, VTt[:, j * KC : (j + 1) * KC], ident
                        )
                        nc.scalar.copy(Vnat[:, j, :], vps)

            # ---- Phase C: causal attention in S^T layout ----
            with ExitStack() as ap:
                asb = ap.enter_context(tc.tile_pool(name="asb", bufs=3))
                asb2 = ap.enter_context(tc.tile_pool(name="asb2", bufs=2))
                aps = ap.enter_context(tc.tile_pool(name="aps", bufs=2, space="PSUM"))
                for g in range(G):
                    for i in range(NQC):
                        q0 = g * T + i * QC
                        nk = (i + 1) * (QC // KC)
                        yps = aps.tile([P, QC], F32, tag="yps")
                        rps = aps.tile([P, QC], F32, tag="rps")
                        for j in range(nk):
                            dcol = max(0, j * KC - i * QC)
                            sps = aps.tile([P, QC], F32, tag="sps")
                            nc.tensor.matmul(
                                sps[:, dcol:QC],
                                KTr[:, j * KC : (j + 1) * KC],
                                QTr[:, q0 + dcol : q0 + QC],
                                start=True,
                                stop=True,
                            )
                            if j * KC >= i * QC:  # diagonal block: causal mask
                                nc.vector.tensor_add(
                                    sps[:, dcol : dcol + KC],
                                    sps[:, dcol : dcol + KC],
                                    trineg,
                                )
                            es = asb.tile([P, QC], F32R, tag="es")
                            nc.scalar.activation(
                                es[:, dcol:QC], sps[:, dcol:QC], AF.Exp
                            )
                            nc.tensor.matmul(
                                yps[:, dcol:QC],
                                Vnat[:, j, :],
                                es[:, dcol:QC],
                                start=(j == 0),
                                stop=(j == nk - 1),
                            )
                            nc.tensor.matmul(
                                rps[:, dcol:QC],
                                ones_r,
                                es[:, dcol:QC],
                                start=(j == 0),
                                stop=(j == nk - 1),
                            )
                        rec = asb2.tile([P, QC], F32, tag="rec")
                        nc.vector.reciprocal_approx_fast(out=rec, in_=rps)
                        yo = asb.tile([P, QC], F32, tag="yo")
                        nc.vector.tensor_mul(yo, yps, rec)
                        nc.sync.dma_start(
                            out=YT[g * DH : (g + 1) * DH, i * QC : (i + 1) * QC],
                            in_=yo,
                        )

    nc.finalize()
    return nc


_NC_CACHE = None


def _get_nc():
    global _NC_CACHE
    if _NC_CACHE is None:
        _NC_CACHE = build_nc()
    return _NC_CACHE


def _host_tables():
    inv_freq = 1.0 / (ROPE_BASE ** (np.arange(0, DH, 2, dtype=np.float32) / DH))
    t = np.arange(T, dtype=np.float32)
    freqs = np.outer(t, inv_freq).astype(np.float32)   # (T, 64)
    cosT = np.cos(freqs).T.astype(np.float32)           # (64, T)
    sinT = np.sin(freqs).T.astype(np.float32)
    ropeC = np.concatenate([cosT, cosT], axis=0)        # (128, T)
    ropeS = np.concatenate([sinT, -sinT], axis=0)
    pp = np.arange(KC)[:, None]
    ff = np.arange(KC)[None, :]
    trineg = np.where(pp <= ff, 0.0, NEG).astype(np.float32)
    return np.ascontiguousarray(ropeC), np.ascontiguousarray(ropeS), trineg


def kernel(x, Wq, Wk, Wv):
    x = np.asarray(x, dtype=np.float32)
    Wq = np.asarray(Wq, dtype=np.float32)
    Wk = np.asarray(Wk, dtype=np.float32)
    Wv = np.asarray(Wv, dtype=np.float32)
    ropeC, ropeS, trineg = _host_tables()

    in_maps = []
    for core in range(NCORES):
        b, h = divmod(core, HKV)
        xT = np.ascontiguousarray(x[b].T).reshape(DCH, P, T)
        Wsl = np.concatenate(
            [
                Wq[h * EQ : (h + 1) * EQ],
                Wk[h * DH : (h + 1) * DH],
                Wv[h * DH : (h + 1) * DH],
            ],
            axis=0,
        )                                                # (768, D)
        wT = np.ascontiguousarray(Wsl.T).reshape(DCH, P, ETOT)
        in_maps.append(
            {"xT": xT, "wT": wT, "ropeC": ropeC, "ropeS": ropeS, "trineg": trineg}
        )

    nc = _get_nc()
    res = run_bass_kernel_spmd(nc, in_maps, core_ids=list(range(NCORES)))

    out = np.empty((B, T, H * DH), dtype=np.float32)
    for core in range(NCORES):
        b, h = divmod(core, HKV)
        yT = res.results[core]["yT"]                     # (512, T)
        out[b, :, h * EQ : (h + 1) * EQ] = (
            yT.reshape(G, DH, T).transpose(2, 0, 1).reshape(T, EQ)
        )
    return out


# revision 11
# speedup vs baseline: 1.0647x; 1.0647x over previous
"""MixerAttention (GQA + QK-RMSNorm + RoPE + causal) Trainium2 kernel.

Sharding: 8 cores = batch(2) x kv-head(4). Fully local per core — no collectives.
Each core, for its (batch b, kv head h):
  - projections for its 4 q heads + 1 kv head: W^T.T @ x^T on the PE, with
    x and W pre-transposed on the host so the D contraction lands on
    partitions; inputs are declared float32r in DRAM so the PE runs at
    full rate from plain HWDGE loads
  - QK RMSNorm via the ln/exp rsqrt path (the Rsqrt ACT table is banned),
    with the 1/sqrt(DH) attention scale folded into the q normalization,
    then RoPE on the DVE — both are column-local, so they run per
    512-column chunk inside the projection pipeline
  - causal attention in S^T layout: scores^T tiles (t_k partitions x t_q
    free) take the additive causal mask on diagonal blocks, exp runs on
    2-k-chunk supertiles to amortize ACT overhead, and exp(S^T) feeds
    P@V directly as the matmul moving operand; softmax denominators ride
    a broadcast ones-matmul; normalization on-chip (reciprocal_approx_fast)
The whole computation is software-pipelined over 4 column windows:
projections(n) | rms+rope chains(n) | V transposes(n) | attention(i=n).
Output per core is y^T (4*128, T); the host reassembles (B, T, H*DH).
"""
import sys

sys.path.insert(0, "/opt/trn_rl_repo")
from contextlib import ExitStack

import numpy as np
import concourse.bacc as bacc
import concourse.mybir as mybir
import concourse.tile as tile
from concourse.bass_utils import run_bass_kernel_spmd
from concourse.masks import make_identity

F32 = mybir.dt.float32
F32R = mybir.dt.float32r
AF = mybir.ActivationFunctionType

B, T, D = 2, 2048, 2048
H, HKV, DH = 16, 4, 128
G = H // HKV                    # q heads per kv head (per core)
EPS = 1.1920928955078125e-07
ROPE_BASE = 10000.0
NCORES = 8

P = 128                         # partitions
DCH = D // P                    # 16 d-chunks (contraction)
NT = 4                          # column windows of 512
TC = T // NT                    # 512
EQ = G * DH                     # 512
ETOT = EQ + DH + DH             # 768
QC = 512                        # attention q-chunk == TC
KC = 128                        # attention k-chunk
NKC = T // KC                   # 16
NEG = -1.0e30
MK, MV = G, G + 1               # m-tile indices of k and v rows


def _chain(nc, pools, src, dst, dst0, ln_scale, ln_bias, ropeC, ropeS, ones_r, n, label):
    """Per-512-chunk RMSNorm (ln/exp rsqrt) + RoPE: src (P,TC) fp32 staging
    -> dst[:, dst0:dst0+TC] f32r."""
    sp, cps = pools
    c0 = n * TC
    sq = sp.tile([P, TC], F32R, tag="sq", name=f"sq_{label}")
    nc.scalar.activation(sq, src, AF.Square)
    ssb = cps.tile([P, TC], F32, tag="pj", bufs=2, name=f"ssb_{label}")
    nc.tensor.matmul(ssb, ones_r, sq, start=True, stop=True)
    lnt = sp.tile([P, TC], F32, tag="lnt", name=f"lnt_{label}")
    nc.scalar.activation(lnt, ssb, AF.Ln, scale=ln_scale, bias=ln_bias[:, :])
    rs = sp.tile([P, TC], F32, tag="rs", name=f"rs_{label}")
    nc.scalar.activation(rs, lnt, AF.Exp, scale=-0.5)
    nc.vector.tensor_mul(src, src, rs)
    # rope: dst = x*C + rot(x)*S  (column-local)
    tmp = sp.tile([P, TC], F32, tag="rtmp", name=f"rtmp_{label}")
    nc.vector.tensor_copy(tmp[0 : P // 2, :], src[P // 2 : P, :])
    nc.vector.tensor_copy(tmp[P // 2 : P, :], src[0 : P // 2, :])
    t1 = sp.tile([P, TC], F32, tag="rt1", name=f"rt1_{label}")
    nc.vector.tensor_mul(t1, src, ropeC[:, c0 : c0 + TC])
    nc.vector.tensor_mul(tmp, tmp, ropeS[:, c0 : c0 + TC])
    nc.vector.tensor_add(dst[:, dst0 : dst0 + TC], t1, tmp)


def _body(nc, tc, ctx):
    XT = nc.cur_io["xT"]
    WT = nc.cur_io["wT"]
    RC = nc.cur_io["ropeC"]
    RS_ = nc.cur_io["ropeS"]
    TRI = nc.cur_io["trineg"]
    YT = nc.cur_io["yT"]

    constp = ctx.enter_context(tc.tile_pool(name="const", bufs=1))
    finp = ctx.enter_context(tc.tile_pool(name="final", bufs=1))
    wp = ctx.enter_context(tc.tile_pool(name="wp", bufs=1))
    xp = ctx.enter_context(tc.tile_pool(name="xp", bufs=17))
    stg = ctx.enter_context(tc.tile_pool(name="stg", bufs=8))
    sp = ctx.enter_context(tc.tile_pool(name="sp", bufs=3))
    qsc = ctx.enter_context(tc.tile_pool(name="qsc", bufs=6))
    asb = ctx.enter_context(tc.tile_pool(name="asb", bufs=2))
    asb2 = ctx.enter_context(tc.tile_pool(name="asb2", bufs=2))
    cps = ctx.enter_context(tc.tile_pool(name="cps", bufs=1, space="PSUM"))

    # weights and window-0 x interleaved so the first matmuls unblock fast
    wt = wp.tile([P, DCH, ETOT], F32R, tag="wt")
    xns = {}
    for d in range(DCH):
        nc.sync.dma_start(out=wt[:, d, :], in_=WT[d, :, :])
        xn = xp.tile([P, TC], F32R, tag="xn", name=f"xn_0_{d}")
        nc.sync.dma_start(out=xn, in_=XT[d, :, 0:TC])
        xns[(0, d)] = xn

    trineg = constp.tile([P, KC], F32, tag="trineg")
    nc.sync.dma_start(out=trineg, in_=TRI[:, :])
    ropeC = constp.tile([P, T], F32, tag="ropeC")
    nc.sync.dma_start(out=ropeC, in_=RC[:, :])
    ropeS = constp.tile([P, T], F32, tag="ropeS")
    nc.sync.dma_start(out=ropeS, in_=RS_[:, :])
    ident = constp.tile([P, P], F32, tag="ident")
    make_identity(nc, ident)
    ones_f = constp.tile([P, P], F32, tag="ones_f")
    nc.vector.memset(ones_f, 1.0)
    ones_r = constp.tile([P, P], F32R, tag="ones_r")
    nc.vector.tensor_copy(ones_r, ones_f)
    bq = constp.tile([P, 1], F32, tag="bq")
    nc.vector.memset(bq, float(P) * EPS)
    bk = constp.tile([P, 1], F32, tag="bk")
    nc.vector.memset(bk, EPS)

    KTr = finp.tile([P, T], F32R, tag="KTr")
    Vnat = finp.tile([P, NKC, KC], F32R, tag="Vnat")

    for n in range(NT):
        # prefetch next window's x
        if n + 1 < NT:
            for d in range(DCH):
                xn = xp.tile([P, TC], F32R, tag="xn", name=f"xn_{n+1}_{d}")
                nc.sync.dma_start(
                    out=xn, in_=XT[d, :, (n + 1) * TC : (n + 2) * TC]
                )
                xns[(n + 1, d)] = xn

        # ---- projections for window n: 3 waves of 2 m-tiles ----
        stage = {}
        for wave in ([MK, MV], [0, 1], [2, 3]):
            psl = {
                m: cps.tile([P, TC], F32, tag="pj", bufs=2, name=f"pj_{n}_{m}")
                for m in wave
            }
            for d in range(DCH):
                for m in wave:
                    nc.tensor.matmul(
                        psl[m],
                        wt[:, d, m * P : (m + 1) * P],
                        xns[(n, d)],
                        start=(d == 0),
                        stop=(d == DCH - 1),
                    )
            for m in wave:
                st = stg.tile([P, TC], F32, tag="stage", name=f"st_{n}_{m}")
                nc.scalar.copy(st, psl[m])
                stage[m] = st

        # ---- chains: k first, then V transposes, then q heads ----
        _chain(nc, (sp, cps), stage[MK], KTr, n * TC, 1.0 / P, bk,
               ropeC, ropeS, ones_r, n, f"k{n}")
        for jj in range(4):
            j = 4 * n + jj
            vps = cps.tile([P, TC], F32, tag="rps", bufs=1, name=f"vps_{j}")
            nc.tensor.transpose(
                vps[:, 0:KC], stage[MV][:, jj * KC : (jj + 1) * KC], ident
            )
            nc.scalar.copy(Vnat[:, j, :], vps[:, 0:KC])

        qtr = {}
        for g in range(G):
            qtr[g] = qsc.tile([P, TC], F32R, tag="qtr", name=f"qtr_{g}_{n}")
            _chain(nc, (sp, cps), stage[g], qtr[g], 0, 1.0, bq,
                   ropeC, ropeS, ones_r, n, f"q{g}_{n}")

        # ---- attention for q-chunk i == n, all 4 heads ----
        i = n
        nk = 4 * (i + 1)
        for g in range(G):
            yps = cps.tile([P, QC], F32, tag="yps", bufs=1, name=f"yps_{g}_{i}")
            rps = cps.tile([P, QC], F32, tag="rps", bufs=1, name=f"rps_{g}_{i}")
            for mpair in range(nk // 2):
                j0 = 2 * mpair
                sps = cps.tile(
                    [P, 2 * QC], F32, tag="sps", bufs=2, name=f"sps_{g}_{i}_{mpair}"
                )
                es = asb.tile(
                    [P, 2 * QC], F32R, tag="es", name=f"es_{g}_{i}_{mpair}"
                )
                dc = [0, 0]
                for c in range(2):
                    j = j0 + c
                    dcol = max(0, j * KC - i * QC)
                    dc[c] = dcol
                    nc.tensor.matmul(
                        sps[:, c * QC + dcol : (c + 1) * QC],
                        KTr[:, j * KC : (j + 1) * KC],
                        qtr[g][:, dcol:QC],
                        start=True,
                        stop=True,
                    )
                    if j * KC >= i * QC:  # diagonal: additive causal mask
                        nc.vector.tensor_add(
                            sps[:, c * QC + dcol : c * QC + dcol + KC],
                            sps[:, c * QC + dcol : c * QC + dcol + KC],
                            trineg,
                        )
                # one exp over both chunks (gap cols hold garbage, never read)
                nc.scalar.activation(
                    es[:, dc[0] : 2 * QC], sps[:, dc[0] : 2 * QC], AF.Exp
                )
                for c in range(2):
                    j = j0 + c
                    dcol = dc[c]
                    nc.tensor.matmul(
                        yps[:, dcol:QC],
                        Vnat[:, j, :],
                        es[:, c * QC + dcol : (c + 1) * QC],
                        start=(j == 0),
                        stop=(j == nk - 1),
                    )
                    nc.tensor.matmul(
                        rps[:, dcol:QC],
                        ones_r,
                        es[:, c * QC + dcol : (c + 1) * QC],
                        start=(j == 0),
                        stop=(j == nk - 1),
                    )
            rec = asb2.tile([P, QC], F32, tag="rec", name=f"rec_{g}_{i}")
            nc.vector.reciprocal_approx_fast(out=rec, in_=rps)
            yo = asb.tile([P, QC], F32, tag="yo", name=f"yo_{g}_{i}")
            nc.vector.tensor_mul(yo, yps, rec)
            nc.sync.dma_start(
                out=YT[g * DH : (g + 1) * DH, i * QC : (i + 1) * QC], in_=yo
            )


def _pin_act_table_set():
    """Restrict the ACT table chooser to natural_log_exp_and_others (which
    holds ln/exp/square/copy — every function this kernel uses) so the
    compiled stream has one table load instead of one per ln<->exp switch
    (~1.3us each). Indices of the full set list are preserved."""
    import concourse.hw_specs as hw_specs

    if getattr(bacc, "_act_tables_pinned", False):
        return
    orig = hw_specs.get_activation_tables
    keep = "natural_log_exp_and_others"

    def patched(arch):
        t = orig(arch)
        return {k: (v if k == keep else set()) for k, v in t.items()}

    bacc.get_activation_tables = patched
    bacc._act_tables_pinned = True


def build_nc(reps=1):
    _pin_act_table_set()
    nc = bacc.Bacc(trn_type="TRN2")
    nc.cur_io = {
        "xT": nc.dram_tensor("xT", [DCH, P, T], F32R, kind="ExternalInput"),
        "wT": nc.dram_tensor("wT", [DCH, P, ETOT], F32R, kind="ExternalInput"),
        "ropeC": nc.dram_tensor("ropeC", [P, T], F32, kind="ExternalInput"),
        "ropeS": nc.dram_tensor("ropeS", [P, T], F32, kind="ExternalInput"),
        "trineg": nc.dram_tensor("trineg", [P, KC], F32, kind="ExternalInput"),
        "yT": nc.dram_tensor("yT", [EQ, T], F32, kind="ExternalOutput"),
    }
    with tile.TileContext(nc) as tc:
        for _rep in range(reps):
            with ExitStack() as ctx:
                _body(nc, tc, ctx)
    nc.finalize()
    return nc


_NC_CACHE = None


def _get_nc():
    global _NC_CACHE
    if _NC_CACHE is None:
        _NC_CACHE = build_nc()
    return _NC_CACHE


def _host_tables():
    inv_freq = 1.0 / (ROPE_BASE ** (np.arange(0, DH, 2, dtype=np.float32) / DH))
    t = np.arange(T, dtype=np.float32)
    freqs = np.outer(t, inv_freq).astype(np.float32)    # (T, 64)
    cosT = np.cos(freqs).T.astype(np.float32)            # (64, T)
    sinT = np.sin(freqs).T.astype(np.float32)
    ropeC = np.concatenate([cosT, cosT], axis=0)         # (128, T)
    ropeS = np.concatenate([sinT, -sinT], axis=0)
    pp_ = np.arange(KC)[:, None]
    ff = np.arange(KC)[None, :]
    trineg = np.where(pp_ <= ff, 0.0, NEG).astype(np.float32)
    return np.ascontiguousarray(ropeC), np.ascontiguousarray(ropeS), trineg


def kernel(x, Wq, Wk, Wv):
    x = np.asarray(x, dtype=np.float32)
    Wq = np.asarray(Wq, dtype=np.float32)
    Wk = np.asarray(Wk, dtype=np.float32)
    Wv = np.asarray(Wv, dtype=np.float32)
    ropeC, ropeS, trineg = _host_tables()

    in_maps = []
    for core in range(NCORES):
        b, h = divmod(core, HKV)
        xT = np.ascontiguousarray(x[b].T).reshape(DCH, P, T)
        Wsl = np.concatenate(
            [
                Wq[h * EQ : (h + 1) * EQ],
                Wk[h * DH : (h + 1) * DH],
                Wv[h * DH : (h + 1) * DH],
            ],
            axis=0,
        )                                                 # (768, D)
        wT = np.ascontiguousarray(Wsl.T).reshape(DCH, P, ETOT)
        in_maps.append(
            {"xT": xT, "wT": wT, "ropeC": ropeC, "ropeS": ropeS, "trineg": trineg}
        )

    nc = _get_nc()
    res = run_bass_kernel_spmd(nc, in_maps, core_ids=list(range(NCORES)))

    out = np.empty((B, T, H * DH), dtype=np.float32)
    for core in range(NCORES):
        b, h = divmod(core, HKV)
        yT = res.results[core]["yT"]                      # (512, T)
        out[b, :, h * EQ : (h + 1) * EQ] = (
            yT.reshape(G, DH, T).transpose(2, 0, 1).reshape(T, EQ)
        )
    return out


# revision 19
# speedup vs baseline: 32089.7678x; 30140.7976x over previous
"""MixerAttention (GQA + QK-RMSNorm + RoPE + causal) Trainium2 kernel.

Sharding: 8 cores = batch(2) x kv-head(4). Fully local per core — no collectives.
Each core, for its (batch b, kv head h):
  - projections for its 4 q heads + 1 kv head: W^T.T @ x^T on the PE, with
    x and W pre-transposed on the host so the D contraction lands on
    partitions; inputs are declared float32r in DRAM so the PE runs at
    full rate from plain HWDGE loads
  - QK RMSNorm via the ln/exp rsqrt path (the Rsqrt ACT table is banned),
    with the 1/sqrt(DH) attention scale folded into the q normalization,
    then RoPE on the DVE — both are column-local, so they run per
    512-column chunk inside the projection pipeline
  - causal attention in S^T layout: scores^T tiles (t_k partitions x t_q
    free) take the additive causal mask on diagonal blocks, exp runs on
    2-k-chunk supertiles to amortize ACT overhead, and exp(S^T) feeds
    P@V directly as the matmul moving operand; softmax denominators ride
    a broadcast ones-matmul; normalization on-chip (reciprocal_approx_fast)
The whole computation is software-pipelined over 4 column windows:
projections(n) | rms+rope chains(n) | V transposes(n) | attention(i=n).
Output per core is y^T (4*128, T); the host reassembles (B, T, H*DH).
"""
import sys

sys.path.insert(0, "/opt/trn_rl_repo")
from contextlib import ExitStack

import numpy as np
import concourse.bacc as bacc
import concourse.mybir as mybir
import concourse.tile as tile
from concourse.bass_utils import run_bass_kernel_spmd
from concourse.masks import make_identity

F32 = mybir.dt.float32
F32R = mybir.dt.float32r
AF = mybir.ActivationFunctionType

B, T, D = 2, 2048, 2048
H, HKV, DH = 16, 4, 128
G = H // HKV                    # q heads per kv head (per core)
EPS = 1.1920928955078125e-07
ROPE_BASE = 10000.0
NCORES = 8

P = 128                         # partitions
DCH = D // P                    # 16 d-chunks (contraction)
NT = 4                          # column windows of 512
TC = T // NT                    # 512
EQ = G * DH                     # 512
ETOT = EQ + DH + DH             # 768
QC = 512                        # attention q-chunk == TC
KC = 128                        # attention k-chunk
NKC = T // KC                   # 16
NEG = -1.0e30
MK, MV = G, G + 1               # m-tile indices of k and v rows


def _chain(nc, pools, src, dst, dst0, ln_scale, ln_bias, ropeC, ropeS, ones_r, n, label):
    """Per-512-chunk RMSNorm (ln/exp rsqrt) + RoPE: src (P,TC) fp32 staging
    -> dst[:, dst0:dst0+TC] f32r."""
    sp, cps = pools
    c0 = n * TC
    sq = sp.tile([P, TC], F32R, tag="sq", name=f"sq_{label}")
    nc.scalar.activation(sq, src, AF.Square)
    ssb = cps.tile([P, TC], F32, tag="pj", bufs=2, name=f"ssb_{label}")
    nc.tensor.matmul(ssb, ones_r, sq, start=True, stop=True)
    lnt = sp.tile([P, TC], F32, tag="lnt", name=f"lnt_{label}")
    nc.scalar.activation(lnt, ssb, AF.Ln, scale=ln_scale, bias=ln_bias[:, :])
    rs = sp.tile([P, TC], F32, tag="rs", name=f"rs_{label}")
    nc.scalar.activation(rs, lnt, AF.Exp, scale=-0.5)
    nc.vector.tensor_mul(src, src, rs)
    # rope: dst = x*C + rot(x)*S  (column-local)
    tmp = sp.tile([P, TC], F32, tag="rtmp", name=f"rtmp_{label}")
    nc.vector.tensor_copy(tmp[0 : P // 2, :], src[P // 2 : P, :])
    nc.vector.tensor_copy(tmp[P // 2 : P, :], src[0 : P // 2, :])
    t1 = sp.tile([P, TC], F32, tag="rt1", name=f"rt1_{label}")
    nc.vector.tensor_mul(t1, src, ropeC[:, c0 : c0 + TC])
    nc.vector.tensor_mul(tmp, tmp, ropeS[:, c0 : c0 + TC])
    nc.vector.tensor_add(dst[:, dst0 : dst0 + TC], t1, tmp)


def _body(nc, tc, ctx):
    XT = nc.cur_io["xT"]
    WT = nc.cur_io["wT"]
    RC = nc.cur_io["ropeC"]
    RS_ = nc.cur_io["ropeS"]
    TRI = nc.cur_io["trineg"]
    YT = nc.cur_io["yT"]

    constp = ctx.enter_context(tc.tile_pool(name="const", bufs=1))
    finp = ctx.enter_context(tc.tile_pool(name="final", bufs=1))
    wp = ctx.enter_context(tc.tile_pool(name="wp", bufs=1))
    xp = ctx.enter_context(tc.tile_pool(name="xp", bufs=17))
    stg = ctx.enter_context(tc.tile_pool(name="stg", bufs=8))
    sp = ctx.enter_context(tc.tile_pool(name="sp", bufs=2))
    qsc = ctx.enter_context(tc.tile_pool(name="qsc", bufs=6))
    asb = ctx.enter_context(tc.tile_pool(name="asb", bufs=2))
    asb2 = ctx.enter_context(tc.tile_pool(name="asb2", bufs=2))
    cps = ctx.enter_context(tc.tile_pool(name="cps", bufs=1, space="PSUM"))

    # weights and window-0 x interleaved so the first matmuls unblock fast;
    # x loads in 4-d-chunk supertiles (1 MiB DMAs, few descriptors)
    wt = wp.tile([P, DCH, ETOT], F32R, tag="wt")
    xns = {}

    def load_x(n, d):
        xn = xp.tile([P, TC], F32R, tag="xn", name=f"xn_{n}_{d}")
        nc.sync.dma_start(out=xn, in_=XT[d, :, n * TC : (n + 1) * TC])
        xns[(n, d)] = xn

    for a in range(4):
        nc.sync.dma_start(
            out=wt[:, 4 * a : 4 * a + 4, :],
            in_=WT[4 * a : 4 * a + 4, :, :].rearrange("d p e -> p d e"),
        )
        for d in range(4 * a, 4 * a + 4):
            load_x(0, d)

    trineg = constp.tile([P, KC], F32, tag="trineg")
    nc.sync.dma_start(out=trineg, in_=TRI[:, :])
    ropeC = constp.tile([P, T], F32, tag="ropeC")
    nc.sync.dma_start(out=ropeC, in_=RC[:, :])
    ropeS = constp.tile([P, T], F32, tag="ropeS")
    nc.sync.dma_start(out=ropeS, in_=RS_[:, :])
    ident = constp.tile([P, P], F32, tag="ident")
    make_identity(nc, ident)
    ones_f = constp.tile([P, P], F32, tag="ones_f")
    nc.vector.memset(ones_f, 1.0)
    ones_r = constp.tile([P, P], F32R, tag="ones_r")
    nc.vector.tensor_copy(ones_r, ones_f)
    bq = constp.tile([P, 1], F32, tag="bq")
    nc.vector.memset(bq, float(P) * EPS)
    bk = constp.tile([P, 1], F32, tag="bk")
    nc.vector.memset(bk, EPS)

    KTr = finp.tile([P, T], F32R, tag="KTr")
    Vnat = finp.tile([P, NKC, KC], F32R, tag="Vnat")

    for n in range(NT):
        # prefetch next window's x
        if n + 1 < NT:
            for d in range(DCH):
                load_x(n + 1, d)

        # ---- projections for window n: 3 waves of 2 m-tiles ----
        stage = {}
        for wave in ([MK, MV], [0, 1], [2, 3]):
            psl = {
                m: cps.tile([P, TC], F32, tag="pj", bufs=2, name=f"pj_{n}_{m}")
                for m in wave
            }
            for d in range(DCH):
                for m in wave:
                    nc.tensor.matmul(
                        psl[m],
                        wt[:, d, m * P : (m + 1) * P],
                        xns[(n, d)],
                        start=(d == 0),
                        stop=(d == DCH - 1),
                    )
            for m in wave:
                st = stg.tile([P, TC], F32, tag="stage", name=f"st_{n}_{m}")
                nc.scalar.copy(st, psl[m])
                stage[m] = st

        # ---- chains: k first, then V transposes, then q heads ----
        _chain(nc, (sp, cps), stage[MK], KTr, n * TC, 1.0 / P, bk,
               ropeC, ropeS, ones_r, n, f"k{n}")
        for jj in range(4):
            j = 4 * n + jj
            vps = cps.tile([P, TC], F32, tag="rps", bufs=1, name=f"vps_{j}")
            nc.tensor.transpose(
                vps[:, 0:KC], stage[MV][:, jj * KC : (jj + 1) * KC], ident
            )
            nc.scalar.copy(Vnat[:, j, :], vps[:, 0:KC])

        qtr = {}
        for g in range(G):
            qtr[g] = qsc.tile([P, TC], F32R, tag="qtr", name=f"qtr_{g}_{n}")
            _chain(nc, (sp, cps), stage[g], qtr[g], 0, 1.0, bq,
                   ropeC, ropeS, ones_r, n, f"q{g}_{n}")

        # ---- attention for q-chunk i == n, all 4 heads ----
        i = n
        nk = 4 * (i + 1)
        for g in range(G):
            yps = cps.tile([P, QC], F32, tag="yps", bufs=1, name=f"yps_{g}_{i}")
            rps = cps.tile([P, QC], F32, tag="rps", bufs=1, name=f"rps_{g}_{i}")
            for mpair in range(nk // 2):
                j0 = 2 * mpair
                sps = cps.tile(
                    [P, 2 * QC], F32, tag="sps", bufs=2, name=f"sps_{g}_{i}_{mpair}"
                )
                es = asb.tile(
                    [P, 2 * QC], F32R, tag="es", name=f"es_{g}_{i}_{mpair}"
                )
                dc = [0, 0]
                for c in range(2):
                    j = j0 + c
                    dcol = max(0, j * KC - i * QC)
                    dc[c] = dcol
                    nc.tensor.matmul(
                        sps[:, c * QC + dcol : (c + 1) * QC],
                        KTr[:, j * KC : (j + 1) * KC],
                        qtr[g][:, dcol:QC],
                        start=True,
                        stop=True,
                    )
                    if j * KC >= i * QC:  # diagonal: additive causal mask
                        nc.vector.tensor_add(
                            sps[:, c * QC + dcol : c * QC + dcol + KC],
                            sps[:, c * QC + dcol : c * QC + dcol + KC],
                            trineg,
                        )
                # one exp over both chunks (gap cols hold garbage, never read)
                nc.scalar.activation(
                    es[:, dc[0] : 2 * QC], sps[:, dc[0] : 2 * QC], AF.Exp
                )
                for c in range(2):
                    j = j0 + c
                    dcol = dc[c]
                    nc.tensor.matmul(
                        yps[:, dcol:QC],
                        Vnat[:, j, :],
                        es[:, c * QC + dcol : (c + 1) * QC],
                        start=(j == 0),
                        stop=(j == nk - 1),
                    )
                    nc.tensor.matmul(
                        rps[:, dcol:QC],
                        ones_r,
                        es[:, c * QC + dcol : (c + 1) * QC],
                        start=(j == 0),
                        stop=(j == nk - 1),
                    )
            rec = asb2.tile([P, QC], F32, tag="rec", name=f"rec_{g}_{i}")
            nc.vector.reciprocal_approx_fast(out=rec, in_=rps)
            yo = asb.tile([P, QC], F32, tag="yo", name=f"yo_{g}_{i}")
            nc.vector.tensor_mul(yo, yps, rec)
            nc.sync.dma_start(
                out=YT[g * DH : (g + 1) * DH, i * QC : (i + 1) * QC], in_=yo
            )


def _pin_act_table_set():
    """Restrict the ACT table chooser to natural_log_exp_and_others (which
    holds ln/exp/square/copy — every function this kernel uses) so the
    compiled stream has one table load instead of one per ln<->exp switch
    (~1.3us each). Indices of the full set list are preserved."""
    import concourse.hw_specs as hw_specs

    if getattr(bacc, "_act_tables_pinned", False):
        return
    orig = hw_specs.get_activation_tables
    keep = "natural_log_exp_and_others"

    def patched(arch):
        t = orig(arch)
        return {k: (v if k == keep else set()) for k, v in t.items()}

    bacc.get_activation_tables = patched
    bacc._act_tables_pinned = True


def build_nc(reps=1):
    _pin_act_table_set()
    nc = bacc.Bacc(trn_type="TRN2")
    nc.cur_io = {
        "xT": nc.dram_tensor("xT", [DCH, P, T], F32R, kind="ExternalInput"),
        "wT": nc.dram_tensor("wT", [DCH, P, ETOT], F32R, kind="ExternalInput"),
        "ropeC": nc.dram_tensor("ropeC", [P, T], F32, kind="ExternalInput"),
        "ropeS": nc.dram_tensor("ropeS", [P, T], F32, kind="ExternalInput"),
        "trineg": nc.dram_tensor("trineg", [P, KC], F32, kind="ExternalInput"),
        "yT": nc.dram_tensor("yT", [EQ, T], F32, kind="ExternalOutput"),
    }
    with tile.TileContext(nc) as tc:
        for _rep in range(reps):
            with ExitStack() as ctx:
                _body(nc, tc, ctx)
    nc.finalize()
    return nc


_NC_CACHE = None


def _get_nc():
    global _NC_CACHE
    if _NC_CACHE is None:
        _NC_CACHE = build_nc()
    return _NC_CACHE


def _host_tables():
    inv_freq = 1.0 / (ROPE_BASE ** (np.arange(0, DH, 2, dtype=np.float32) / DH))
    t = np.arange(T, dtype=np.float32)
    freqs = np.outer(t, inv_freq).astype(np.float32)    # (T, 64)
    cosT = np.cos(freqs).T.astype(np.float32)            # (64, T)
    sinT = np.sin(freqs).T.astype(np.float32)
    ropeC = np.concatenate([cosT, cosT], axis=0)         # (128, T)
    ropeS = np.concatenate([sinT, -sinT], axis=0)
    pp_ = np.arange(KC)[:, None]
    ff = np.arange(KC)[None, :]
    trineg = np.where(pp_ <= ff, 0.0, NEG).astype(np.float32)
    return np.ascontiguousarray(ropeC), np.ascontiguousarray(ropeS), trineg


def kernel(x, Wq, Wk, Wv):
    x = np.asarray(x, dtype=np.float32)
    Wq = np.asarray(Wq, dtype=np.float32)
    Wk = np.asarray(Wk, dtype=np.float32)
    Wv = np.asarray(Wv, dtype=np.float32)
    ropeC, ropeS, trineg = _host_tables()

    in_maps = []
    for core in range(NCORES):
        b, h = divmod(core, HKV)
        xT = np.ascontiguousarray(x[b].T).reshape(DCH, P, T)
        Wsl = np.concatenate(
            [
                Wq[h * EQ : (h + 1) * EQ],
                Wk[h * DH : (h + 1) * DH],
                Wv[h * DH : (h + 1) * DH],
            ],
            axis=0,
        )                                                 # (768, D)
        wT = np.ascontiguousarray(Wsl.T).reshape(DCH, P, ETOT)
        in_maps.append(
            {"xT": xT, "wT": wT, "ropeC": ropeC, "ropeS": ropeS, "trineg": trineg}
        )

    nc = _get_nc()
    res = run_bass_kernel_spmd(nc, in_maps, core_ids=list(range(NCORES)))

    out = np.empty((B, T, H * DH), dtype=np.float32)
    for core in range(NCORES):
        b, h = divmod(core, HKV)
        yT = res.results[core]["yT"]                      # (512, T)
        out[b, :, h * EQ : (h + 1) * EQ] = (
            yT.reshape(G, DH, T).transpose(2, 0, 1).reshape(T, EQ)
        )
    return out
